# revision 12
# baseline (speedup 1.0000x reference)
"""Trainium2 Bass kernel for AlignNet — pure data-parallel over batch (64 -> 8x8).

Self-contained: hardcodes the architecture, shards inputs across 8 NeuronCores,
runs one SPMD NEFF (built once, cached), gathers full outputs.

Layout convention on device: activations live in SBUF as [C, B, Lpad] tiles
(channels on partitions, batch-major free dim, padded length).  Convs are
K-tap matmul accumulations into PSUM (contraction over channel groups), with
bias+ReLU fused into the PSUM->SBUF epilogue.  Linear upsampling and the
grid-sample warp are expressed as small matmuls against host-precomputed
(static) interpolation matrices / on-device-built bilinear weight matrices.
"""

import numpy as np

import concourse.bass as bass
import concourse.bacc as bacc
import concourse.mybir as mybir
from concourse.tile import TileContext
from concourse.bass_utils import run_bass_kernel_spmd

F32 = mybir.dt.float32
F32R = mybir.dt.float32r
AF = mybir.ActivationFunctionType
ALU = mybir.AluOpType

B = 8            # per-core batch
NCORES = 8
L0 = 512
LMEL = 2056
ATT_MAP = np.array([0, 0, 0, 1, 1, 1, 2, 2, 2, 2, 3, 3, 3, 3, 4, 4, 4, 4, 0, 1],
                   dtype=np.int32)

LV = [170, 85, 43, 22]     # video pyramid lengths
LA = [64, 32, 16, 8]       # audio pyramid lengths
VUP = [512, 170, 85, 43]   # up_flow target length per level

_VP0 = [(5, 3, 1), (3, 1, 1), (3, 1, 1), (3, 1, 1), (3, 1, 1)]
_AP0 = [(7, 4, 4), (5, 2, 1), (5, 2, 1), (3, 2, 1), (3, 1, 1)]
_PN = [(3, 2, 1), (3, 1, 1), (3, 1, 1)]

# upsample matrices: name -> (Lf, Lo, scale)
_UPS = [
    ("u3", 22, 43, 1.0), ("u2", 43, 85, 1.0), ("u1", 85, 170, 1.0),
    ("u0", 170, 512, 1.0),
    ("ua2", 22, 16, 7.5), ("ua1", 43, 32, 15.5), ("ua0", 85, 64, 31.5),
]
_GBS = [("gb2", 16), ("gb1", 32), ("gb0", 64)]


def _olen(L, k, s, p, d=1):
    return (L + 2 * p - d * (k - 1) - 1) // s + 1


def _make_spec():
    """Layer registry: lid -> dict(gs, cout, k, s, p, d, relu)."""
    layers = {}

    def add(lid, gs, cout, k, s, p, d=1, relu=True):
        layers[lid] = dict(gs=list(gs), cout=cout, k=k, s=s, p=p, d=d, relu=relu)

    for pre in ("v", "a"):
        for pi in range(4):
            if pi == 0:
                specs = _VP0 if pre == "v" else _AP0
                cin, cout = 80, 128
            else:
                specs = _PN
                cin, cout = 128 >> (pi - 1), 128 >> pi
            c = cin
            for j, (k, s, p) in enumerate(specs):
                add(f"{pre}{pi}c{j}", [c], cout, k, s, p)
                c = cout
    for i in range(4):
        la, cv, cadd = LA[i], 128 >> i, 32 >> i
        gs = [la, cv] + ([1] if i != 3 else [])
        add(f"e{i}a", gs, cadd, 3, 1, 1)
        add(f"e{i}b", gs + [cadd], cadd, 3, 1, 1)
        add(f"p{i}", gs + [cadd, cadd], 1, 3, 1, 1, relu=False)
    dpgs = [64, 128, 1, 32, 32]
    cin, cout, j = 257, 64, 0
    while cin > 1:
        add(f"dp{j}", dpgs if j == 0 else [cin], cout, 3, 1, 2 ** j, 2 ** j)
        cin, cout, j = cout, max(cout // 4, 1), j + 1
    return layers


def _layout():
    """Column layout of the packed consts tensor [128, ncols]."""
    spec = _make_spec()
    meta = {"w": {}, "b": {}, "U": {}, "gb": {}}
    col = [0]

    def alloc(n):
        c0 = col[0]
        col[0] += n
        return c0

    for lid, sp in spec.items():
        for gi in range(len(sp["gs"])):
            for dk in range(sp["k"]):
                meta["w"][(lid, gi, dk)] = alloc(sp["cout"])
        meta["b"][lid] = alloc(1)
    for name, lf, lo, _sc in _UPS:
        nch = (lf + 127) // 128
        lo_eff = lo + (lo & 1)
        meta["U"][name] = [(min(128, lf - 128 * c), alloc(lo_eff))
                           for c in range(nch)]
    for name, la in _GBS:
        meta["gb"][name] = alloc(1)
    meta["ones"] = alloc(64)
    meta["ncols"] = col[0]
    return spec, meta


def _upmat(lf, lo):
    """Linear-interp upsample matrix mirroring torch Upsample(align_corners=False).

    pos computed in float32 to match the jax reference."""
    pos = (np.arange(lo, dtype=np.float32) + np.float32(0.5)) * np.float32(lf / lo) \
        - np.float32(0.5)
    pos = np.clip(pos, np.float32(0.0), np.float32(lf - 1))
    lo_i = np.floor(pos).astype(np.int32)
    hi_i = np.minimum(lo_i + 1, lf - 1)
    w = (pos - lo_i.astype(np.float32)).astype(np.float64)
    U = np.zeros((lf, lo), np.float64)
    idx = np.arange(lo)
    np.add.at(U, (lo_i, idx), 1.0 - w)
    np.add.at(U, (hi_i, idx), w)
    return U


def _pack_consts(params, spec, meta):
    M = np.zeros((128, meta["ncols"]), np.float32)

    def put(lid, w, b):
        sp = spec[lid]
        w = np.asarray(w, np.float32)
        off = 0
        for gi, g in enumerate(sp["gs"]):
            for dk in range(sp["k"]):
                c0 = meta["w"][(lid, gi, dk)]
                M[0:g, c0:c0 + sp["cout"]] = w[:, off:off + g, dk].T
            off += g
        M[0:sp["cout"], meta["b"][lid]] = np.asarray(b, np.float32)

    # softmax(kp_att)[ATT_MAP] folded into the first video conv's weights
    ka = np.asarray(params["kp_att"], np.float64)
    e = np.exp(ka - ka.max())
    att = (e / e.sum())[ATT_MAP]            # [20] per keypoint
    attc = np.repeat(att, 4).astype(np.float64)  # [80] per input channel

    for pi in range(4):
        for j, p in enumerate(params["video_pyrs"][pi]):
            w = np.asarray(p["w"], np.float64)
            if pi == 0 and j == 0:
                w = w * attc[None, :, None]
            put(f"v{pi}c{j}", w, p["b"])
        for j, p in enumerate(params["audio_pyrs"][pi]):
            put(f"a{pi}c{j}", p["w"], p["b"])
    for i in range(4):
        put(f"e{i}a", params["extractors"][i][0]["w"], params["extractors"][i][0]["b"])
        put(f"e{i}b", params["extractors"][i][1]["w"], params["extractors"][i][1]["b"])
        put(f"p{i}", params["predictors"][i]["w"], params["predictors"][i]["b"])
    for j, p in enumerate(params["dp"]):
        put(f"dp{j}", p["w"], p["b"])

    for name, lf, lo, sc in _UPS:
        U = _upmat(lf, lo) * sc
        for (rows, c0), r0 in zip(meta["U"][name], range(0, lf, 128)):
            M[0:rows, c0:c0 + lo] = U[r0:r0 + rows].astype(np.float32)
    for name, la in _GBS:
        M[0:la, meta["gb"][name]] = (0.5 * (la - 1) - np.arange(la)).astype(np.float32)
    M[0, meta["ones"]:meta["ones"] + 64] = 1.0
    return M


def _bc(ap):
    return ap.bitcast(F32R)


# debug: subset of {"video", "audio", "decoder"} to emit (decoder needs both)
_STAGES = {"video", "audio", "decoder"}


def _build():
    spec, meta = _layout()
    nc = bacc.Bacc()
    vf_in = nc.declare_dram_parameter("vf", [80, B, 514], F32, isOutput=False)
    af_in = nc.declare_dram_parameter("af", [80, B, 2072], F32, isOutput=False)
    cst_in = nc.declare_dram_parameter("consts", [128, meta["ncols"]], F32,
                                       isOutput=False)
    out_d = [nc.declare_dram_parameter(f"out{k}", [B, n], F32, isOutput=True)
             for k, n in enumerate([512, 170, 85, 43])]

    with TileContext(nc) as tc:
        with tc.tile_pool(name="mp", bufs=1) as mp, \
             tc.tile_pool(name="pp", bufs=7, space="PSUM") as pp, \
             tc.tile_pool(name="dpool", bufs=1, space="DRAM") as dpool:

            consts = mp.tile([128, meta["ncols"]], F32, name="consts_sb")
            ncol = meta["ncols"]
            nchunk = 12
            step = (ncol + nchunk - 1) // nchunk
            for ci in range(nchunk):
                a, b2 = ci * step, min((ci + 1) * step, ncol)
                nc.sync.dma_start(consts[:, a:b2].bitcast(F32R),
                                  cst_in[:, a:b2].bitcast(F32R))

            def new_act(pool, name, C, Lint, pad):
                """[C, B, pad+Lint+pad+4] tile with zeroed pads (padr=pad+4
                so fp32r even-count matmuls can read one column past Lout)."""
                t = pool.tile([C, B, 2 * pad + Lint + 4], F32, name=name)
                if pad:
                    nc.gpsimd.memset(t[0:C, :, 0:pad], 0.0)
                nc.gpsimd.memset(t[0:C, :, pad + Lint:], 0.0)
                return t

            def conv(lid, groups, Lout, out_t, out_pad, epi="act"):
                """groups: list of (ap3, csize, padl). Writes out interior."""
                sp = spec[lid]
                k, s, p, d, cout, relu = (sp["k"], sp["s"], sp["p"], sp["d"],
                                          sp["cout"], sp["relu"])
                nmm = len(groups) * k
                bias = consts[0:cout, meta["b"][lid]:meta["b"][lid] + 1]
                Leff = Lout + (Lout & 1)
                nb = max(1, 512 // Leff)
                for b0 in range(0, B, nb):
                    nbb = min(nb, B - b0)
                    ps = pp.tile([cout, nbb, Leff], F32, name=f"{lid}_ps",
                                 tag="ps", bufs=7)
                    i = 0
                    for gi, (gap, cs, gpadl) in enumerate(groups):
                        for dk in range(k):
                            c0 = meta["w"][(lid, gi, dk)]
                            loff = gpadl + dk * d - p
                            rhs = gap[0:cs, b0:b0 + nbb,
                                      loff:loff + (Leff - 1) * s + 1:s]
                            nc.tensor.matmul(
                                ps[:, :, :],
                                lhsT=_bc(consts[0:cs, c0:c0 + cout]),
                                rhs=_bc(rhs),
                                start=(i == 0), stop=(i == nmm - 1))
                            i += 1
                    o = out_t[0:cout, b0:b0 + nbb,
                              out_pad:out_pad + Lout].bitcast(F32R)
                    pss = ps[:, :, 0:Lout]
                    if epi == "act":
                        nc.scalar.activation(o, pss,
                                             AF.Relu if relu else AF.Identity,
                                             bias=bias)
                    else:
                        if relu:
                            nc.vector.tensor_scalar(o, pss, bias, 0.0,
                                                    ALU.add, ALU.max)
                        else:
                            nc.vector.tensor_scalar_add(o, pss, bias)

            # ---------------- pyramids ----------------
            vfeat = [None] * 4
            afeat = [None] * 4
            with tc.tile_pool(name="pyr", bufs=1) as wp:
              if "video" in _STAGES:
                # video
                vin = wp.tile([80, B, 514], F32, name="vin")
                nc.sync.dma_start(vin[:, :, :].bitcast(F32R),
                                  vf_in[:, :, :].bitcast(F32R))
                cur, cpad = vin, 1
                for j in range(5):
                    Lo = LV[0]
                    if j == 4:
                        nxt = new_act(mp, "vf0", 128, Lo, 1)
                    else:
                        nxt = new_act(wp, f"v0_{j}", 128, Lo, 1)
                    conv(f"v0c{j}", [(cur, 80 if j == 0 else 128, cpad)], Lo,
                         nxt, 1, epi="act")
                    cur, cpad = nxt, 1
                vfeat[0] = cur
                for pi in range(1, 4):
                    cin = 128 >> (pi - 1)
                    cout = 128 >> pi
                    for j in range(3):
                        Lo = LV[pi]
                        if j == 2:
                            nxt = new_act(mp, f"vf{pi}", cout, Lo, 1)
                        else:
                            nxt = new_act(wp, f"v{pi}_{j}", cout, Lo, 1)
                        conv(f"v{pi}c{j}", [(cur, cin if j == 0 else cout, cpad)],
                             Lo, nxt, 1, epi="act")
                        cur, cpad = nxt, 1
                    vfeat[pi] = cur

              if "audio" in _STAGES:
                # audio conv1 (per-sample streaming, Lout=515 split in two)
                a0_1 = new_act(wp, "a0c0o", 128, 515, 1)
                lid = "a0c0"
                bias0 = consts[0:128, meta["b"][lid]:meta["b"][lid] + 1]
                for bb in range(B):
                    ainb = wp.tile([80, 1, 2072], F32, name="ainb", tag="ainb",
                                   bufs=3)
                    nc.sync.dma_start(ainb[:, :, :].bitcast(F32R),
                                      af_in[0:80, bb:bb + 1, :].bitcast(F32R))
                    for (l0, nmm_l, nout) in ((0, 258, 258), (258, 258, 257)):
                        ps = pp.tile([128, 1, nmm_l], F32, name="a0c0_ps",
                                     tag="ps", bufs=7)
                        for dk in range(7):
                            c0 = meta["w"][(lid, 0, dk)]
                            loff = l0 * 4 + dk
                            rhs = ainb[0:80, 0:1,
                                       loff:loff + (nmm_l - 1) * 4 + 1:4]
                            nc.tensor.matmul(
                                ps[:, :, :],
                                lhsT=_bc(consts[0:80, c0:c0 + 128]),
                                rhs=_bc(rhs), start=(dk == 0), stop=(dk == 6))
                        nc.vector.tensor_scalar(
                            a0_1[0:128, bb:bb + 1,
                                 1 + l0:1 + l0 + nout].bitcast(F32R),
                            ps[:, :, 0:nout], bias0, 0.0, ALU.add, ALU.max)
                # audio conv2..5
                alens = [515, 257, 128, 64, 64]
                cur, cpad = a0_1, 1
                for j in range(1, 5):
                    Lo = alens[j]
                    if j == 4:
                        nxt = new_act(mp, "af0", 128, Lo, 1)
                    else:
                        nxt = new_act(wp, f"a0_{j}", 128, Lo, 1)
                    conv(f"a0c{j}", [(cur, 128, cpad)], Lo, nxt, 1, epi="dve")
                    cur, cpad = nxt, 1
                afeat[0] = cur
                for pi in range(1, 4):
                    cin = 128 >> (pi - 1)
                    cout = 128 >> pi
                    for j in range(3):
                        Lo = LA[pi]
                        if j == 2:
                            nxt = new_act(mp, f"af{pi}", cout, Lo, 1)
                        else:
                            nxt = new_act(wp, f"a{pi}_{j}", cout, Lo, 1)
                        conv(f"a{pi}c{j}", [(cur, cin if j == 0 else cout, cpad)],
                             Lo, nxt, 1, epi="dve")
                        cur, cpad = nxt, 1
                    afeat[pi] = cur

            # ---------------- decoder ----------------
            if "decoder" in _STAGES:
             with tc.tile_pool(name="dec", bufs=1) as dw:

                def transposed_flow(i, flow_sb, Lf):
                    """flow [1,B,Lf] -> list of [rows, B] sbuf chunks (via DRAM)."""
                    fb = dpool.tile([Lf, B], F32, name=f"fb{i}")
                    nc.sync.dma_start(
                        fb[:, :].rearrange("l b -> b l").bitcast(F32R),
                        flow_sb[0:1, :, :].bitcast(F32R))
                    fts = []
                    for c in range((Lf + 127) // 128):
                        rows = min(128, Lf - 128 * c)
                        ft = dw.tile([rows, B], F32, name=f"ft{i}_{c}")
                        nc.sync.dma_start(ft[:, :].bitcast(F32R),
                                          fb[128 * c:128 * c + rows, :].bitcast(F32R))
                        fts.append((ft, rows))
                    return fts

                def upsample(fts, uname, lo):
                    lo_eff = lo + (lo & 1)
                    ps = pp.tile([B, lo_eff], F32, name=f"up_{uname}", tag="ps",
                                 bufs=7)
                    chunks = meta["U"][uname]
                    for ci, ((rows, c0), (ft, rows2)) in enumerate(
                            zip(chunks, fts)):
                        nc.tensor.matmul(
                            ps[:, :], lhsT=_bc(ft[0:rows, 0:B]),
                            rhs=_bc(consts[0:rows, c0:c0 + lo_eff]),
                            start=(ci == 0), stop=(ci == len(chunks) - 1))
                    return ps

                def emit_corr(i, G, corr_t):
                    Ca = 128 >> i
                    la, lv = LA[i], LV[i]
                    lveff = lv + (lv & 1)
                    for bb in range(B):
                        cfp = pp.tile([la, lveff], F32, name="cfp", tag="ps",
                                      bufs=7)
                        nc.tensor.matmul(
                            cfp[:, :], lhsT=_bc(afeat[i][0:Ca, bb, 1:1 + la]),
                            rhs=_bc(vfeat[i][0:Ca, bb, 1:1 + lveff]),
                            start=True, stop=True)
                        if G is None:
                            nc.vector.tensor_copy(
                                corr_t[0:la, bb, 1:1 + lv].bitcast(F32R),
                                cfp[:, 0:lv])
                            continue
                        cfs = dw.tile([la, lveff], F32, name="cfs", tag="cfs",
                                      bufs=4)
                        nc.vector.tensor_copy(cfs[:, :].bitcast(F32R),
                                              cfp[:, :])
                        crp = pp.tile([la, lveff], F32, name="crp", tag="ps",
                                      bufs=7)
                        nc.tensor.matmul(crp[:, :],
                                         lhsT=_bc(G[0:la, bb, 0:la]),
                                         rhs=_bc(cfs[:, :]),
                                         start=True, stop=True)
                        nc.vector.tensor_copy(
                            corr_t[0:la, bb, 1:1 + lv].bitcast(F32R),
                            crp[:, 0:lv])

                upch = {}
                Gs = {}

                def emit_flow_products(i, flow_sb):
                    """After flow_i: video upsample -> out_i (+ upch[i-1]),
                    audio upsample -> G[i-1]."""
                    fts = transposed_flow(i, flow_sb, LV[i])
                    lo = VUP[i]
                    ups = upsample(fts, f"u{i}", lo)
                    usb = dw.tile([B, lo], F32, name=f"usb{i}", tag="usb",
                                  bufs=2)
                    nc.scalar.copy(usb[:, :], ups[:, 0:lo])
                    nc.sync.dma_start(out_d[i][:, :], usb[:, :])
                    if i == 0:
                        return
                    j = i - 1
                    uc = new_act(dw, f"upch{j}", 1, lo, 1)
                    nc.sync.dma_start(uc[0:1, :, 1:1 + lo].bitcast(F32R),
                                      usb[0:B, 0:lo].bitcast(F32R))
                    upch[j] = uc
                    la = LA[j]
                    ua = upsample(fts, f"ua{j}", la)
                    ixs = dw.tile([B, la], F32, name=f"ixs{j}")
                    nc.scalar.copy(ixs[:, :], ua[:, 0:la])
                    ixt = dw.tile([1, B, la], F32, name=f"ixt{j}")
                    nc.sync.dma_start(ixt[0:1, :, :].bitcast(F32R),
                                      ixs[0:B, 0:la].bitcast(F32R))
                    gps = pp.tile([la, B, la], F32, name=f"gps{j}", tag="ps",
                                  bufs=7)
                    oc = meta["ones"]
                    nc.tensor.matmul(gps[:, :, :],
                                     lhsT=_bc(consts[0:1, oc:oc + la]),
                                     rhs=_bc(ixt[0:1, :, :]),
                                     start=True, stop=True)
                    gt = dw.tile([la, B, la], F32, name="gtmp", tag="gtmp",
                                 bufs=2)
                    gbc = meta["gb"][f"gb{j}"]
                    nc.scalar.activation(gt[:, :, :], gps[:, :, :], AF.Abs,
                                         bias=consts[0:la, gbc:gbc + 1])
                    G = dw.tile([la, B, la], F32, name=f"G{j}")
                    nc.scalar.activation(G[:, :, :].bitcast(F32R),
                                         gt[:, :, :], AF.Relu,
                                         bias=1.0, scale=-1.0)
                    Gs[j] = G

                flows = {}
                for i in (3, 2, 1, 0):
                    la, cv, cadd = LA[i], 128 >> i, 32 >> i
                    corr_t = new_act(dw, f"corr{i}", la, LV[i], 1)
                    emit_corr(i, Gs.get(i), corr_t)
                    groups = [(corr_t, la, 1), (vfeat[i], cv, 1)]
                    if i != 3:
                        groups.append((upch[i], 1, 1))
                    ea = new_act(dw, f"e{i}a_t", cadd, LV[i], 1)
                    conv(f"e{i}a", groups, LV[i], ea, 1, epi="act")
                    groups.append((ea, cadd, 1))
                    eb = new_act(dw, f"e{i}b_t", cadd, LV[i], 1)
                    conv(f"e{i}b", groups, LV[i], eb, 1, epi="act")
                    groups.append((eb, cadd, 1))
                    fl = dw.tile([1, B, LV[i]], F32, name=f"flow{i}")
                    conv(f"p{i}", groups, LV[i], fl, 0, epi="act")
                    flows[i] = fl
                    if i != 0:
                        emit_flow_products(i, fl)
                    else:
                        feat0_groups = groups

                # dp chain
                dpads = [2, 4, 8, 0]
                douts = [64, 16, 4, 1]
                cur_groups = feat0_groups
                for j in range(4):
                    pad = dpads[j]
                    t = new_act(dw, f"dpb{j}", douts[j], 170, pad)
                    conv(f"dp{j}", cur_groups, 170, t, pad, epi="dve")
                    cur_groups = [(t, douts[j], pad)]
                dp4 = cur_groups[0][0]
                flF = dw.tile([1, B, 170], F32, name="flF")
                nc.vector.tensor_add(flF[0:1, :, :], flows[0][0:1, :, :],
                                     dp4[0:1, :, 0:170])
                emit_flow_products(0, flF)

    nc.finalize()
    return nc, spec, meta


_CACHE = {}


def _get_graph():
    if "nc" not in _CACHE:
        nc, spec, meta = _build()
        _CACHE["nc"] = (nc, spec, meta)
    return _CACHE["nc"]


def _prep_core_inputs(vf8, af8, consts):
    """vf8 [8,512,20,2,2], af8 [8,80,2056] -> device layouts."""
    v = np.ascontiguousarray(
        vf8.reshape(B, 512, 80).transpose(2, 0, 1)).astype(np.float32)
    vp = np.zeros((80, B, 514), np.float32)
    vp[:, :, 1:513] = v
    a = np.ascontiguousarray(af8.transpose(1, 0, 2)).astype(np.float32)
    ap_ = np.zeros((80, B, 2072), np.float32)
    ap_[:, :, 4:2060] = a
    return {"vf": vp, "af": ap_, "consts": consts}


def kernel(video_feature, audio_feature, params):
    nc, spec, meta = _get_graph()
    consts = _pack_consts(params, spec, meta)
    vf = np.asarray(video_feature, np.float32)
    af = np.asarray(audio_feature, np.float32)
    in_maps = [
        _prep_core_inputs(vf[c * B:(c + 1) * B], af[c * B:(c + 1) * B], consts)
        for c in range(NCORES)
    ]
    res = run_bass_kernel_spmd(nc, in_maps, core_ids=list(range(NCORES)))
    outs = tuple(
        np.concatenate([res.results[c][f"out{k}"] for c in range(NCORES)], axis=0)
        for k in range(4))
    return outs


# revision 14
# speedup vs baseline: 9.1752x; 9.1752x over previous
"""Trainium2 Bass kernel for AlignNet — pure data-parallel over batch (64 -> 8x8).

Self-contained: hardcodes the architecture, shards inputs across 8 NeuronCores,
runs one SPMD NEFF (built once, cached), gathers full outputs.

Layout convention on device: activations live in SBUF as [C, B, Lpad] tiles
(channels on partitions, batch-major free dim, padded length).  Convs are
K-tap matmul accumulations into PSUM (contraction over channel groups), with
bias+ReLU fused into the PSUM->SBUF epilogue.  Linear upsampling and the
grid-sample warp are expressed as small matmuls against host-precomputed
(static) interpolation matrices / on-device-built bilinear weight matrices.
"""

import numpy as np

import concourse.bass as bass
import concourse.bacc as bacc
import concourse.mybir as mybir
from concourse.tile import TileContext
from concourse.bass_utils import run_bass_kernel_spmd

F32 = mybir.dt.float32
F32R = mybir.dt.float32r
AF = mybir.ActivationFunctionType
ALU = mybir.AluOpType

B = 8            # per-core batch
NCORES = 8
L0 = 512
LMEL = 2056
ATT_MAP = np.array([0, 0, 0, 1, 1, 1, 2, 2, 2, 2, 3, 3, 3, 3, 4, 4, 4, 4, 0, 1],
                   dtype=np.int32)

LV = [170, 85, 43, 22]     # video pyramid lengths
LA = [64, 32, 16, 8]       # audio pyramid lengths
VUP = [512, 170, 85, 43]   # up_flow target length per level

_VP0 = [(5, 3, 1), (3, 1, 1), (3, 1, 1), (3, 1, 1), (3, 1, 1)]
_AP0 = [(7, 4, 4), (5, 2, 1), (5, 2, 1), (3, 2, 1), (3, 1, 1)]
_PN = [(3, 2, 1), (3, 1, 1), (3, 1, 1)]

# upsample matrices: name -> (Lf, Lo, scale)
_UPS = [
    ("u3", 22, 43, 1.0), ("u2", 43, 85, 1.0), ("u1", 85, 170, 1.0),
    ("u0", 170, 512, 1.0),
    ("ua2", 22, 16, 7.5), ("ua1", 43, 32, 15.5), ("ua0", 85, 64, 31.5),
]
_GBS = [("gb2", 16), ("gb1", 32), ("gb0", 64)]


def _olen(L, k, s, p, d=1):
    return (L + 2 * p - d * (k - 1) - 1) // s + 1


def _make_spec():
    """Layer registry: lid -> dict(gs, cout, k, s, p, d, relu)."""
    layers = {}

    def add(lid, gs, cout, k, s, p, d=1, relu=True):
        layers[lid] = dict(gs=list(gs), cout=cout, k=k, s=s, p=p, d=d, relu=relu)

    for pre in ("v", "a"):
        for pi in range(4):
            if pi == 0:
                specs = _VP0 if pre == "v" else _AP0
                cin, cout = 80, 128
            else:
                specs = _PN
                cin, cout = 128 >> (pi - 1), 128 >> pi
            c = cin
            for j, (k, s, p) in enumerate(specs):
                add(f"{pre}{pi}c{j}", [c], cout, k, s, p)
                c = cout
    for i in range(4):
        la, cv, cadd = LA[i], 128 >> i, 32 >> i
        gs = [la, cv] + ([1] if i != 3 else [])
        add(f"e{i}a", gs, cadd, 3, 1, 1)
        add(f"e{i}b", gs + [cadd], cadd, 3, 1, 1)
        add(f"p{i}", gs + [cadd, cadd], 1, 3, 1, 1, relu=False)
    dpgs = [64, 128, 1, 32, 32]
    cin, cout, j = 257, 64, 0
    while cin > 1:
        add(f"dp{j}", dpgs if j == 0 else [cin], cout, 3, 1, 2 ** j, 2 ** j)
        cin, cout, j = cout, max(cout // 4, 1), j + 1
    return layers


def _layout():
    """Column layout of the packed consts tensor [128, ncols]."""
    spec = _make_spec()
    meta = {"w": {}, "b": {}, "U": {}, "gb": {}}
    col = [0]

    def alloc(n):
        c0 = col[0]
        col[0] += n
        return c0

    for lid, sp in spec.items():
        for gi in range(len(sp["gs"])):
            for dk in range(sp["k"]):
                meta["w"][(lid, gi, dk)] = alloc(sp["cout"])
        meta["b"][lid] = alloc(1)
    for name, lf, lo, _sc in _UPS:
        nch = (lf + 127) // 128
        lo_eff = lo + (lo & 1)
        meta["U"][name] = [(min(128, lf - 128 * c), alloc(lo_eff))
                           for c in range(nch)]
    for name, la in _GBS:
        meta["gb"][name] = alloc(1)
    meta["ones"] = alloc(64)
    meta["ncols"] = col[0]
    return spec, meta


def _upmat(lf, lo):
    """Linear-interp upsample matrix mirroring torch Upsample(align_corners=False).

    pos computed in float32 to match the jax reference."""
    pos = (np.arange(lo, dtype=np.float32) + np.float32(0.5)) * np.float32(lf / lo) \
        - np.float32(0.5)
    pos = np.clip(pos, np.float32(0.0), np.float32(lf - 1))
    lo_i = np.floor(pos).astype(np.int32)
    hi_i = np.minimum(lo_i + 1, lf - 1)
    w = (pos - lo_i.astype(np.float32)).astype(np.float64)
    U = np.zeros((lf, lo), np.float64)
    idx = np.arange(lo)
    np.add.at(U, (lo_i, idx), 1.0 - w)
    np.add.at(U, (hi_i, idx), w)
    return U


def _pack_consts(params, spec, meta):
    M = np.zeros((128, meta["ncols"]), np.float32)

    def put(lid, w, b):
        sp = spec[lid]
        w = np.asarray(w, np.float32)
        off = 0
        for gi, g in enumerate(sp["gs"]):
            for dk in range(sp["k"]):
                c0 = meta["w"][(lid, gi, dk)]
                M[0:g, c0:c0 + sp["cout"]] = w[:, off:off + g, dk].T
            off += g
        M[0:sp["cout"], meta["b"][lid]] = np.asarray(b, np.float32)

    # softmax(kp_att)[ATT_MAP] folded into the first video conv's weights
    ka = np.asarray(params["kp_att"], np.float64)
    e = np.exp(ka - ka.max())
    att = (e / e.sum())[ATT_MAP]            # [20] per keypoint
    attc = np.repeat(att, 4).astype(np.float64)  # [80] per input channel

    for pi in range(4):
        for j, p in enumerate(params["video_pyrs"][pi]):
            w = np.asarray(p["w"], np.float64)
            if pi == 0 and j == 0:
                w = w * attc[None, :, None]
            put(f"v{pi}c{j}", w, p["b"])
        for j, p in enumerate(params["audio_pyrs"][pi]):
            put(f"a{pi}c{j}", p["w"], p["b"])
    for i in range(4):
        put(f"e{i}a", params["extractors"][i][0]["w"], params["extractors"][i][0]["b"])
        put(f"e{i}b", params["extractors"][i][1]["w"], params["extractors"][i][1]["b"])
        put(f"p{i}", params["predictors"][i]["w"], params["predictors"][i]["b"])
    for j, p in enumerate(params["dp"]):
        put(f"dp{j}", p["w"], p["b"])

    for name, lf, lo, sc in _UPS:
        U = _upmat(lf, lo) * sc
        for (rows, c0), r0 in zip(meta["U"][name], range(0, lf, 128)):
            M[0:rows, c0:c0 + lo] = U[r0:r0 + rows].astype(np.float32)
    for name, la in _GBS:
        M[0:la, meta["gb"][name]] = (0.5 * (la - 1) - np.arange(la)).astype(np.float32)
    M[0, meta["ones"]:meta["ones"] + 64] = 1.0
    return M


def _bc(ap):
    return ap.bitcast(F32R)


# debug: subset of {"video", "audio", "decoder"} to emit (decoder needs both)
_STAGES = {"video", "audio", "decoder"}
_LOOP_N = 1  # >1: wrap whole body in a For_i for HW timing


def _build():
    spec, meta = _layout()
    nc = bacc.Bacc()
    vf_in = nc.declare_dram_parameter("vf", [80, B, 514], F32, isOutput=False)
    af_in = nc.declare_dram_parameter("af", [80, B, 2072], F32, isOutput=False)
    cst_in = nc.declare_dram_parameter("consts", [128, meta["ncols"]], F32,
                                       isOutput=False)
    out_d = [nc.declare_dram_parameter(f"out{k}", [B, n], F32, isOutput=True)
             for k, n in enumerate([512, 170, 85, 43])]

    import contextlib
    with TileContext(nc) as tc:
        _st = contextlib.ExitStack()
        if _LOOP_N > 1:
            _st.enter_context(tc.For_i(0, _LOOP_N, 1))
        with _st, \
             tc.tile_pool(name="mp", bufs=1) as mp, \
             tc.tile_pool(name="pp", bufs=7, space="PSUM") as pp, \
             tc.tile_pool(name="dpool", bufs=1, space="DRAM") as dpool:

            consts = mp.tile([128, meta["ncols"]], F32, name="consts_sb")
            ncol = meta["ncols"]
            nchunk = 12
            step = (ncol + nchunk - 1) // nchunk
            for ci in range(nchunk):
                a, b2 = ci * step, min((ci + 1) * step, ncol)
                nc.sync.dma_start(consts[:, a:b2].bitcast(F32R),
                                  cst_in[:, a:b2].bitcast(F32R))

            def new_act(pool, name, C, Lint, pad):
                """[C, B, pad+Lint+pad+4] tile with zeroed pads (padr=pad+4
                so fp32r even-count matmuls can read one column past Lout)."""
                t = pool.tile([C, B, 2 * pad + Lint + 4], F32, name=name)
                if pad:
                    nc.gpsimd.memset(t[0:C, :, 0:pad], 0.0)
                nc.gpsimd.memset(t[0:C, :, pad + Lint:], 0.0)
                return t

            def conv(lid, groups, Lout, out_t, out_pad, epi="act"):
                """groups: list of (ap3, csize, padl). Writes out interior."""
                sp = spec[lid]
                k, s, p, d, cout, relu = (sp["k"], sp["s"], sp["p"], sp["d"],
                                          sp["cout"], sp["relu"])
                nmm = len(groups) * k
                bias = consts[0:cout, meta["b"][lid]:meta["b"][lid] + 1]
                Leff = Lout + (Lout & 1)
                nb = max(1, 512 // Leff)
                for b0 in range(0, B, nb):
                    nbb = min(nb, B - b0)
                    ps = pp.tile([cout, nbb, Leff], F32, name=f"{lid}_ps",
                                 tag="ps", bufs=7)
                    i = 0
                    for gi, (gap, cs, gpadl) in enumerate(groups):
                        for dk in range(k):
                            c0 = meta["w"][(lid, gi, dk)]
                            loff = gpadl + dk * d - p
                            rhs = gap[0:cs, b0:b0 + nbb,
                                      loff:loff + (Leff - 1) * s + 1:s]
                            nc.tensor.matmul(
                                ps[:, :, :],
                                lhsT=_bc(consts[0:cs, c0:c0 + cout]),
                                rhs=_bc(rhs),
                                start=(i == 0), stop=(i == nmm - 1))
                            i += 1
                    o = out_t[0:cout, b0:b0 + nbb,
                              out_pad:out_pad + Lout].bitcast(F32R)
                    pss = ps[:, :, 0:Lout]
                    if epi == "act":
                        nc.scalar.activation(o, pss,
                                             AF.Relu if relu else AF.Identity,
                                             bias=bias)
                    else:
                        if relu:
                            nc.vector.tensor_scalar(o, pss, bias, 0.0,
                                                    ALU.add, ALU.max)
                        else:
                            nc.vector.tensor_scalar_add(o, pss, bias)

            # ---------------- pyramids ----------------
            vfeat = [None] * 4
            afeat = [None] * 4
            with tc.tile_pool(name="pyr", bufs=1) as wp:
              if "video" in _STAGES:
                # video
                vin = wp.tile([80, B, 514], F32, name="vin")
                nc.sync.dma_start(vin[:, :, :].bitcast(F32R),
                                  vf_in[:, :, :].bitcast(F32R))
                cur, cpad = vin, 1
                for j in range(5):
                    Lo = LV[0]
                    if j == 4:
                        nxt = new_act(mp, "vf0", 128, Lo, 1)
                    else:
                        nxt = new_act(wp, f"v0_{j}", 128, Lo, 1)
                    conv(f"v0c{j}", [(cur, 80 if j == 0 else 128, cpad)], Lo,
                         nxt, 1, epi="act")
                    cur, cpad = nxt, 1
                vfeat[0] = cur
                for pi in range(1, 4):
                    cin = 128 >> (pi - 1)
                    cout = 128 >> pi
                    for j in range(3):
                        Lo = LV[pi]
                        if j == 2:
                            nxt = new_act(mp, f"vf{pi}", cout, Lo, 1)
                        else:
                            nxt = new_act(wp, f"v{pi}_{j}", cout, Lo, 1)
                        conv(f"v{pi}c{j}", [(cur, cin if j == 0 else cout, cpad)],
                             Lo, nxt, 1, epi="act")
                        cur, cpad = nxt, 1
                    vfeat[pi] = cur

              if "audio" in _STAGES:
                # audio conv1 (per-sample streaming, Lout=515 split in two)
                a0_1 = new_act(wp, "a0c0o", 128, 515, 1)
                lid = "a0c0"
                bias0 = consts[0:128, meta["b"][lid]:meta["b"][lid] + 1]
                for bb in range(B):
                    ainb = wp.tile([80, 1, 2072], F32, name="ainb", tag="ainb",
                                   bufs=3)
                    nc.sync.dma_start(ainb[:, :, :].bitcast(F32R),
                                      af_in[0:80, bb:bb + 1, :].bitcast(F32R))
                    for (l0, nmm_l, nout) in ((0, 258, 258), (258, 258, 257)):
                        ps = pp.tile([128, 1, nmm_l], F32, name="a0c0_ps",
                                     tag="ps", bufs=7)
                        for dk in range(7):
                            c0 = meta["w"][(lid, 0, dk)]
                            loff = l0 * 4 + dk
                            rhs = ainb[0:80, 0:1,
                                       loff:loff + (nmm_l - 1) * 4 + 1:4]
                            nc.tensor.matmul(
                                ps[:, :, :],
                                lhsT=_bc(consts[0:80, c0:c0 + 128]),
                                rhs=_bc(rhs), start=(dk == 0), stop=(dk == 6))
                        nc.vector.tensor_scalar(
                            a0_1[0:128, bb:bb + 1,
                                 1 + l0:1 + l0 + nout].bitcast(F32R),
                            ps[:, :, 0:nout], bias0, 0.0, ALU.add, ALU.max)
                # audio conv2..5
                alens = [515, 257, 128, 64, 64]
                cur, cpad = a0_1, 1
                for j in range(1, 5):
                    Lo = alens[j]
                    if j == 4:
                        nxt = new_act(mp, "af0", 128, Lo, 1)
                    else:
                        nxt = new_act(wp, f"a0_{j}", 128, Lo, 1)
                    conv(f"a0c{j}", [(cur, 128, cpad)], Lo, nxt, 1, epi="dve")
                    cur, cpad = nxt, 1
                afeat[0] = cur
                for pi in range(1, 4):
                    cin = 128 >> (pi - 1)
                    cout = 128 >> pi
                    for j in range(3):
                        Lo = LA[pi]
                        if j == 2:
                            nxt = new_act(mp, f"af{pi}", cout, Lo, 1)
                        else:
                            nxt = new_act(wp, f"a{pi}_{j}", cout, Lo, 1)
                        conv(f"a{pi}c{j}", [(cur, cin if j == 0 else cout, cpad)],
                             Lo, nxt, 1, epi="dve")
                        cur, cpad = nxt, 1
                    afeat[pi] = cur

            # ---------------- decoder ----------------
            if "decoder" in _STAGES:
             with tc.tile_pool(name="dec", bufs=1) as dw:

                def transposed_flow(i, flow_sb, Lf):
                    """flow [1,B,Lf] -> list of [rows, B] sbuf chunks (via DRAM)."""
                    fb = dpool.tile([Lf, B], F32, name=f"fb{i}")
                    nc.sync.dma_start(
                        fb[:, :].rearrange("l b -> b l").bitcast(F32R),
                        flow_sb[0:1, :, :].bitcast(F32R))
                    fts = []
                    for c in range((Lf + 127) // 128):
                        rows = min(128, Lf - 128 * c)
                        ft = dw.tile([rows, B], F32, name=f"ft{i}_{c}")
                        nc.sync.dma_start(ft[:, :].bitcast(F32R),
                                          fb[128 * c:128 * c + rows, :].bitcast(F32R))
                        fts.append((ft, rows))
                    return fts

                def upsample(fts, uname, lo):
                    lo_eff = lo + (lo & 1)
                    ps = pp.tile([B, lo_eff], F32, name=f"up_{uname}", tag="ps",
                                 bufs=7)
                    chunks = meta["U"][uname]
                    for ci, ((rows, c0), (ft, rows2)) in enumerate(
                            zip(chunks, fts)):
                        nc.tensor.matmul(
                            ps[:, :], lhsT=_bc(ft[0:rows, 0:B]),
                            rhs=_bc(consts[0:rows, c0:c0 + lo_eff]),
                            start=(ci == 0), stop=(ci == len(chunks) - 1))
                    return ps

                def emit_corr(i, G, corr_t):
                    Ca = 128 >> i
                    la, lv = LA[i], LV[i]
                    lveff = lv + (lv & 1)
                    for bb in range(B):
                        cfp = pp.tile([la, lveff], F32, name="cfp", tag="ps",
                                      bufs=7)
                        nc.tensor.matmul(
                            cfp[:, :], lhsT=_bc(afeat[i][0:Ca, bb, 1:1 + la]),
                            rhs=_bc(vfeat[i][0:Ca, bb, 1:1 + lveff]),
                            start=True, stop=True)
                        if G is None:
                            nc.vector.tensor_copy(
                                corr_t[0:la, bb, 1:1 + lv].bitcast(F32R),
                                cfp[:, 0:lv])
                            continue
                        cfs = dw.tile([la, lveff], F32, name="cfs", tag="cfs",
                                      bufs=4)
                        nc.vector.tensor_copy(cfs[:, :].bitcast(F32R),
                                              cfp[:, :])
                        crp = pp.tile([la, lveff], F32, name="crp", tag="ps",
                                      bufs=7)
                        nc.tensor.matmul(crp[:, :],
                                         lhsT=_bc(G[0:la, bb, 0:la]),
                                         rhs=_bc(cfs[:, :]),
                                         start=True, stop=True)
                        nc.vector.tensor_copy(
                            corr_t[0:la, bb, 1:1 + lv].bitcast(F32R),
                            crp[:, 0:lv])

                upch = {}
                Gs = {}

                def emit_flow_products(i, flow_sb):
                    """After flow_i: video upsample -> out_i (+ upch[i-1]),
                    audio upsample -> G[i-1]."""
                    fts = transposed_flow(i, flow_sb, LV[i])
                    lo = VUP[i]
                    ups = upsample(fts, f"u{i}", lo)
                    usb = dw.tile([B, lo], F32, name=f"usb{i}", tag="usb",
                                  bufs=2)
                    nc.scalar.copy(usb[:, :], ups[:, 0:lo])
                    nc.sync.dma_start(out_d[i][:, :], usb[:, :])
                    if i == 0:
                        return
                    j = i - 1
                    uc = new_act(dw, f"upch{j}", 1, lo, 1)
                    nc.sync.dma_start(uc[0:1, :, 1:1 + lo].bitcast(F32R),
                                      usb[0:B, 0:lo].bitcast(F32R))
                    upch[j] = uc
                    la = LA[j]
                    ua = upsample(fts, f"ua{j}", la)
                    ixs = dw.tile([B, la], F32, name=f"ixs{j}")
                    nc.scalar.copy(ixs[:, :], ua[:, 0:la])
                    ixt = dw.tile([1, B, la], F32, name=f"ixt{j}")
                    nc.sync.dma_start(ixt[0:1, :, :].bitcast(F32R),
                                      ixs[0:B, 0:la].bitcast(F32R))
                    gps = pp.tile([la, B, la], F32, name=f"gps{j}", tag="ps",
                                  bufs=7)
                    oc = meta["ones"]
                    nc.tensor.matmul(gps[:, :, :],
                                     lhsT=_bc(consts[0:1, oc:oc + la]),
                                     rhs=_bc(ixt[0:1, :, :]),
                                     start=True, stop=True)
                    gt = dw.tile([la, B, la], F32, name="gtmp", tag="gtmp",
                                 bufs=2)
                    gbc = meta["gb"][f"gb{j}"]
                    nc.scalar.activation(gt[:, :, :], gps[:, :, :], AF.Abs,
                                         bias=consts[0:la, gbc:gbc + 1])
                    G = dw.tile([la, B, la], F32, name=f"G{j}")
                    nc.scalar.activation(G[:, :, :].bitcast(F32R),
                                         gt[:, :, :], AF.Relu,
                                         bias=1.0, scale=-1.0)
                    Gs[j] = G

                flows = {}
                for i in (3, 2, 1, 0):
                    la, cv, cadd = LA[i], 128 >> i, 32 >> i
                    corr_t = new_act(dw, f"corr{i}", la, LV[i], 1)
                    emit_corr(i, Gs.get(i), corr_t)
                    groups = [(corr_t, la, 1), (vfeat[i], cv, 1)]
                    if i != 3:
                        groups.append((upch[i], 1, 1))
                    ea = new_act(dw, f"e{i}a_t", cadd, LV[i], 1)
                    conv(f"e{i}a", groups, LV[i], ea, 1, epi="act")
                    groups.append((ea, cadd, 1))
                    eb = new_act(dw, f"e{i}b_t", cadd, LV[i], 1)
                    conv(f"e{i}b", groups, LV[i], eb, 1, epi="act")
                    groups.append((eb, cadd, 1))
                    fl = dw.tile([1, B, LV[i]], F32, name=f"flow{i}")
                    conv(f"p{i}", groups, LV[i], fl, 0, epi="act")
                    flows[i] = fl
                    if i != 0:
                        emit_flow_products(i, fl)
                    else:
                        feat0_groups = groups

                # dp chain
                dpads = [2, 4, 8, 0]
                douts = [64, 16, 4, 1]
                cur_groups = feat0_groups
                for j in range(4):
                    pad = dpads[j]
                    t = new_act(dw, f"dpb{j}", douts[j], 170, pad)
                    conv(f"dp{j}", cur_groups, 170, t, pad, epi="dve")
                    cur_groups = [(t, douts[j], pad)]
                dp4 = cur_groups[0][0]
                flF = dw.tile([1, B, 170], F32, name="flF")
                nc.vector.tensor_add(flF[0:1, :, :], flows[0][0:1, :, :],
                                     dp4[0:1, :, 0:170])
                emit_flow_products(0, flF)

    nc.finalize()
    return nc, spec, meta


_CACHE = {}


def _get_graph():
    if "nc" not in _CACHE:
        nc, spec, meta = _build()
        _CACHE["nc"] = (nc, spec, meta)
    return _CACHE["nc"]


def _prep_core_inputs(vf8, af8, consts):
    """vf8 [8,512,20,2,2], af8 [8,80,2056] -> device layouts."""
    v = np.ascontiguousarray(
        vf8.reshape(B, 512, 80).transpose(2, 0, 1)).astype(np.float32)
    vp = np.zeros((80, B, 514), np.float32)
    vp[:, :, 1:513] = v
    a = np.ascontiguousarray(af8.transpose(1, 0, 2)).astype(np.float32)
    ap_ = np.zeros((80, B, 2072), np.float32)
    ap_[:, :, 4:2060] = a
    return {"vf": vp, "af": ap_, "consts": consts}


def kernel(video_feature, audio_feature, params):
    nc, spec, meta = _get_graph()
    consts = _pack_consts(params, spec, meta)
    vf = np.asarray(video_feature, np.float32)
    af = np.asarray(audio_feature, np.float32)
    in_maps = [
        _prep_core_inputs(vf[c * B:(c + 1) * B], af[c * B:(c + 1) * B], consts)
        for c in range(NCORES)
    ]
    res = run_bass_kernel_spmd(nc, in_maps, core_ids=list(range(NCORES)))
    outs = tuple(
        np.concatenate([res.results[c][f"out{k}"] for c in range(NCORES)], axis=0)
        for k in range(4))
    return outs


# revision 42
# speedup vs baseline: 24.9878x; 2.7234x over previous
"""Trainium2 Bass kernel for AlignNet — pure data-parallel over batch (64 -> 8x8).

Self-contained: hardcodes the architecture, shards inputs across 8 NeuronCores,
runs one SPMD NEFF (built once, cached), gathers full outputs.

Device layout: activations live in SBUF as [C, B, Lpad] tiles (channels on
partitions, batch-major free dim, padded length).  Convs are K-tap matmul
accumulations into PSUM (contraction over channel groups) with bias+ReLU fused
into the PSUM->SBUF epilogue.  Linear upsampling and the grid-sample warp are
small matmuls against host-precomputed interpolation matrices / on-device
bilinear weight matrices (relu(1-|ix-la|)).  All matmuls run in fp32r
(tf32-like, 1 cycle/row at N>=256), which requires even innermost counts and
fp32r-typed producers.

Per decoder level the correlation + both extractor outputs share one packed
"extras" tile (rows [corr | ea | eb]) so the predictor/dp convs contract 3
channel groups instead of 5.
"""

import contextlib

import numpy as np

import concourse.bass as bass
import concourse.bacc as bacc
import concourse.mybir as mybir
from concourse.tile import TileContext
from concourse.bass_utils import run_bass_kernel_spmd

F32 = mybir.dt.float32
F32R = mybir.dt.float32r
AF = mybir.ActivationFunctionType
ALU = mybir.AluOpType

B = 8            # per-core batch
NCORES = 8
ATT_MAP = np.array([0, 0, 0, 1, 1, 1, 2, 2, 2, 2, 3, 3, 3, 3, 4, 4, 4, 4, 0, 1],
                   dtype=np.int32)

LV = [170, 85, 43, 22]     # video pyramid lengths
LA = [64, 32, 16, 8]       # audio pyramid lengths
VUP = [512, 170, 85, 43]   # up_flow target length per level

_VP0 = [(5, 3, 1), (3, 1, 1), (3, 1, 1), (3, 1, 1), (3, 1, 1)]
_AP0 = [(7, 4, 4), (5, 2, 1), (5, 2, 1), (3, 2, 1), (3, 1, 1)]
_PN = [(3, 2, 1), (3, 1, 1), (3, 1, 1)]

# upsample matrices: name -> (Lf, Lo, scale)
_UPS = [
    ("u3", 22, 43, 1.0), ("u2", 43, 85, 1.0), ("u1", 85, 170, 1.0),
    ("u0", 170, 512, 1.0),
    ("ua2", 22, 16, 7.5), ("ua1", 43, 32, 15.5), ("ua0", 85, 64, 31.5),
]
_GBS = [("gb2", 16), ("gb1", 32), ("gb0", 64)]


def _make_spec():
    """Layer registry: lid -> dict(gs, cout, k, s, p, d, relu).

    gs is a list of channel groups; each group is a list of (offset, size)
    slices of the layer's logical input-channel axis (reference order).  A
    group's slices are packed contiguously into one stationary operand, so
    one matmul contracts the whole group."""
    layers = {}

    def add(lid, gs, cout, k, s, p, d=1, relu=True):
        norm = []
        for g in gs:
            if isinstance(g, int):
                norm.append([(0, g)])
            elif isinstance(g, tuple) and g[0] == "tap3":
                norm.append(g)
            else:
                norm.append(list(g))
        layers[lid] = dict(gs=norm, cout=cout, k=k, s=s, p=p, d=d, relu=relu)

    for pre in ("a", "v"):    # audio first: it is the critical chain
        for pi in range(4):
            if pi == 0:
                specs = _AP0 if pre == "a" else _VP0
                cin, cout = 80, 128
            else:
                specs = _PN
                cin, cout = 128 >> (pi - 1), 128 >> pi
            c = cin
            for j, (k, s, p) in enumerate(specs):
                add(f"{pre}{pi}c{j}", [c], cout, k, s, p)
                c = cout
    for i in range(4):
        la, cv, cadd = LA[i], 128 >> i, 32 >> i
        up = [] if i == 3 else [[(la + cv, 1)]]
        # logical channel axis: [corr(la) | v(cv) | up(0/1) | ea(cadd) | eb(cadd)]
        # pack tile rows: corr@0, ea@AE (32-aligned), eb@BE (32-aligned),
        # with zero-weighted gap rows in between.
        upo = 0 if i == 3 else 1
        AE = max(32, la)
        BE = -(-(AE + cadd) // 32) * 32
        ea_off, eb_off = la + cv + upo, la + cv + upo + cadd
        gap1 = [(None, AE - la)] if AE > la else []
        gap2 = [(None, BE - AE - cadd)] if BE > AE + cadd else []
        # the 1-channel up_flow group is "tap3": its operand tile holds the
        # channel 3x, pre-shifted by one column per tap, so all three taps
        # contract in a single matmul (weight block rows = the 3 taps)
        upg = [] if i == 3 else [("tap3", la + cv)]
        add(f"e{i}a", [[(0, la)], [(la, cv)]] + upg, cadd, 3, 1, 1)
        add(f"e{i}b",
            [[(0, la)] + gap1 + [(ea_off, cadd)], [(la, cv)]] + upg,
            cadd, 3, 1, 1)
        pk = [[(0, la)] + gap1 + [(ea_off, cadd)] + gap2 + [(eb_off, cadd)],
              [(la, cv)]] + upg
        add(f"p{i}", pk, 1, 3, 1, 1, relu=False)
        if i == 0:
            add("dp0", pk, 64, 3, 1, 1)
    cin, cout, j = 64, 16, 1
    while cin > 1:
        add(f"dp{j}", [cin], cout, 3, 1, 2 ** j, 2 ** j)
        cin, cout, j = cout, max(cout // 4, 1), j + 1
    return layers


def _layout():
    """Column layout of the packed consts tensor [128, ncols]."""
    spec = _make_spec()
    meta = {"w": {}, "b": {}, "U": {}, "gb": {}}
    col = [0]

    def alloc(n):
        c0 = col[0]
        col[0] += n
        return c0

    for lid, sp in spec.items():
        for gi, g in enumerate(sp["gs"]):
            ndk = 1 if (isinstance(g, tuple) and g[0] == "tap3") else sp["k"]
            for dk in range(ndk):
                meta["w"][(lid, gi, dk)] = alloc(sp["cout"])
        meta["b"][lid] = alloc(1)
    for name, lf, lo, _sc in _UPS:
        nch = (lf + 127) // 128
        lo_eff = lo + (lo & 1)
        meta["U"][name] = [(min(128, lf - 128 * c), alloc(lo_eff))
                           for c in range(nch)]
    for name, la in _GBS:
        meta["gb"][name] = alloc(1)
    meta["ones"] = alloc(64)
    meta["ncols"] = col[0]
    return spec, meta


def _upmat(lf, lo):
    """Linear-interp upsample matrix mirroring torch Upsample(align_corners=
    False); pos computed in float32 to match the jax reference."""
    pos = (np.arange(lo, dtype=np.float32) + np.float32(0.5)) * np.float32(lf / lo) \
        - np.float32(0.5)
    pos = np.clip(pos, np.float32(0.0), np.float32(lf - 1))
    lo_i = np.floor(pos).astype(np.int32)
    hi_i = np.minimum(lo_i + 1, lf - 1)
    w = (pos - lo_i.astype(np.float32)).astype(np.float64)
    U = np.zeros((lf, lo), np.float64)
    idx = np.arange(lo)
    np.add.at(U, (lo_i, idx), 1.0 - w)
    np.add.at(U, (hi_i, idx), w)
    return U


def _pack_consts(params, spec, meta):
    M = np.zeros((128, meta["ncols"]), np.float32)

    def put(lid, w, b):
        sp = spec[lid]
        w = np.asarray(w, np.float32)
        for gi, slices in enumerate(sp["gs"]):
            if isinstance(slices, tuple) and slices[0] == "tap3":
                ch = slices[1]
                c0 = meta["w"][(lid, gi, 0)]
                for dk in range(sp["k"]):
                    M[dk, c0:c0 + sp["cout"]] = w[:, ch, dk]
                continue
            for dk in range(sp["k"]):
                c0 = meta["w"][(lid, gi, dk)]
                r0 = 0
                for (off, size) in slices:
                    if off is not None:
                        M[r0:r0 + size, c0:c0 + sp["cout"]] = \
                            w[:, off:off + size, dk].T
                    r0 += size
        M[0:sp["cout"], meta["b"][lid]] = np.asarray(b, np.float32)

    # softmax(kp_att)[ATT_MAP] folded into the first video conv's weights
    ka = np.asarray(params["kp_att"], np.float64)
    e = np.exp(ka - ka.max())
    att = (e / e.sum())[ATT_MAP]
    attc = np.repeat(att, 4).astype(np.float64)

    for pi in range(4):
        for j, p in enumerate(params["video_pyrs"][pi]):
            w = np.asarray(p["w"], np.float64)
            if pi == 0 and j == 0:
                w = w * attc[None, :, None]
            put(f"v{pi}c{j}", w, p["b"])
        for j, p in enumerate(params["audio_pyrs"][pi]):
            put(f"a{pi}c{j}", p["w"], p["b"])
    for i in range(4):
        put(f"e{i}a", params["extractors"][i][0]["w"], params["extractors"][i][0]["b"])
        put(f"e{i}b", params["extractors"][i][1]["w"], params["extractors"][i][1]["b"])
        put(f"p{i}", params["predictors"][i]["w"], params["predictors"][i]["b"])
    for j, p in enumerate(params["dp"]):
        put(f"dp{j}", p["w"], p["b"])

    for name, lf, lo, sc in _UPS:
        U = _upmat(lf, lo) * sc
        for (rows, c0), r0 in zip(meta["U"][name], range(0, lf, 128)):
            M[0:rows, c0:c0 + lo] = U[r0:r0 + rows].astype(np.float32)
    for name, la in _GBS:
        M[0:la, meta["gb"][name]] = (0.5 * (la - 1) - np.arange(la)).astype(np.float32)
    M[0, meta["ones"]:meta["ones"] + 64] = 1.0
    return M


def _bc(ap):
    return ap.bitcast(F32R)


_STAGES = {"video", "audio", "decoder"}
_LOOP_N = 1  # >1: wrap whole body in a For_i for HW timing


def _build():
    spec, meta = _layout()
    nc = bacc.Bacc()
    vf_in = nc.declare_dram_parameter("vf", [80, B, 514], F32, isOutput=False)
    af_in = nc.declare_dram_parameter("af", [80, B, 2072], F32, isOutput=False)
    cst_in = nc.declare_dram_parameter("consts", [128, meta["ncols"]], F32,
                                       isOutput=False)
    out_d = [nc.declare_dram_parameter(f"out{k}", [B, n], F32, isOutput=True)
             for k, n in enumerate([512, 170, 85, 43])]

    with TileContext(nc) as tc:
        _st = contextlib.ExitStack()
        if _LOOP_N > 1:
            _st.enter_context(tc.For_i(0, _LOOP_N, 1))
        with _st, \
             tc.tile_pool(name="mp", bufs=1) as mp, \
             tc.tile_pool(name="pp", bufs=7, space="PSUM") as pp:

            consts = mp.tile([128, meta["ncols"]], F32, name="consts_sb")
            ncol = meta["ncols"]
            nchunk = 16
            step = (ncol + nchunk - 1) // nchunk

            def consts_chunk(ci, eng):
                a, b2 = ci * step, min((ci + 1) * step, ncol)
                eng.dma_start(consts[:, a:b2].bitcast(F32R),
                              cst_in[:, a:b2].bitcast(F32R))

            consts_chunk(0, nc.sync)
            consts_chunk(1, nc.scalar)

            def new_act(pool, name, C, Lint, pad, padr_extra=0):
                """[C, B, pad+Lint+pad+4(+extra)] tile with zeroed pads
                (padr >= pad+4 so fp32r even-count matmuls may read past
                Lout; extra for free-dim-padded small convs)."""
                t = pool.tile([C, B, 2 * pad + Lint + 4 + padr_extra], F32,
                              name=name)
                if pad:
                    nc.gpsimd.memset(t[0:C, :, 0:pad], 0.0)
                nc.gpsimd.memset(t[0:C, :, pad + Lint:], 0.0)
                return t

            def conv(lid, groups, Lout, out_t, out_pad, epi="act", out_r0=0,
                     out_lmajor=False, b_range=None, max_nb=8,
                     force_leff=None):
                """groups: list of (ap3, row0, csize, padl) aligned with the
                layer's gs. Writes out_t rows [out_r0:out_r0+cout]."""
                sp = spec[lid]
                k, s, p, d, cout, relu = (sp["k"], sp["s"], sp["p"], sp["d"],
                                          sp["cout"], sp["relu"])
                nmm = sum(1 if isinstance(g[0], str) else k for g in groups)
                bias = consts[0:cout, meta["b"][lid]:meta["b"][lid] + 1]
                Leff = force_leff or (Lout + (Lout & 1))
                nb = min(max(1, 512 // Leff), max_nb)
                tile_list = ([b_range] if b_range is not None else
                             [(b0, min(nb, B - b0))
                              for b0 in range(0, B, nb)])
                for (b0, nbb) in tile_list:
                    ps = pp.tile([cout, nbb, Leff], F32, name=f"{lid}_ps",
                                 tag="ps", bufs=7)
                    i = 0
                    for gi, grp in enumerate(groups):
                        if isinstance(grp[0], str):
                            _, gap, gpadl = grp
                            c0 = meta["w"][(lid, gi, 0)]
                            rhs = gap[0:3, b0:b0 + nbb,
                                      gpadl:gpadl + (Leff - 1) * s + 1:s]
                            nc.tensor.matmul(
                                ps[:, :, :],
                                lhsT=_bc(consts[0:3, c0:c0 + cout]),
                                rhs=_bc(rhs),
                                start=(i == 0), stop=(i == nmm - 1))
                            i += 1
                            continue
                        gap, r0, cs, gpadl = grp
                        for dk in range(k):
                            c0 = meta["w"][(lid, gi, dk)]
                            loff = gpadl + dk * d - p
                            rhs = gap[r0:r0 + cs, b0:b0 + nbb,
                                      loff:loff + (Leff - 1) * s + 1:s]
                            nc.tensor.matmul(
                                ps[:, :, :],
                                lhsT=_bc(consts[0:cs, c0:c0 + cout]),
                                rhs=_bc(rhs),
                                start=(i == 0), stop=(i == nmm - 1))
                            i += 1
                    if out_lmajor:
                        o = out_t[out_r0:out_r0 + cout,
                                  out_pad:out_pad + Lout,
                                  b0:b0 + nbb].rearrange("c l b -> c b l")
                    else:
                        o = out_t[out_r0:out_r0 + cout, b0:b0 + nbb,
                                  out_pad:out_pad + Lout]
                    o = o.bitcast(F32R)
                    pss = ps[:, :, 0:Lout]
                    if epi == "act":
                        nc.scalar.activation(o, pss,
                                             AF.Relu if relu else AF.Identity,
                                             bias=bias)
                    else:
                        if relu:
                            nc.vector.tensor_scalar(o, pss, bias, 0.0,
                                                    ALU.add, ALU.max)
                        else:
                            nc.vector.tensor_scalar_add(o, pss, bias)

            def conv_tiles(Lout):
                Leff = Lout + (Lout & 1)
                nb = max(1, 512 // Leff)
                return [(b0, min(nb, B - b0)) for b0 in range(0, B, nb)]

            # ---------------- pyramids (interleaved audio/video) -------
            vfeat = [None] * 4
            afeat = [None] * 4
            # decoder pack tiles + stored correlation tiles live in mp so the
            # warp-independent correlations can be computed mid-pyramid
            pks = {}
            PKA = {}
            for i in range(4):
                la, cadd = LA[i], 32 >> i
                AE = max(32, la)
                BE = -(-(AE + cadd) // 32) * 32
                pk = new_act(mp, f"pk{i}", BE + cadd, LV[i], 1,
                             padr_extra=8 if i == 3 else 0)
                if AE > la:
                    nc.gpsimd.memset(pk[0:AE, :, :], 0.0)
                if BE > AE + cadd:
                    nc.gpsimd.memset(pk[AE:BE, :, :], 0.0)
                pks[i] = pk
                PKA[i] = (AE, BE)
            cfss = {}

            def emit_cf(i):
                """Warp-independent per-sample correlations a_i x v_i.
                i==3: written straight into pk3 (no warp at level 3)."""
                Ca = 128 >> i
                la, lv = LA[i], LV[i]
                lveff = 256 if i == 0 else lv + (lv & 1)
                for bb in range(B):
                    cfp = pp.tile([la, lveff], F32, name="cfp", tag="ps",
                                  bufs=7)
                    nc.tensor.matmul(
                        cfp[:, :], lhsT=_bc(afeat[i][0:Ca, bb, 1:1 + la]),
                        rhs=_bc(vfeat[i][0:Ca, bb, 1:1 + lveff]),
                        start=True, stop=True)
                    if i == 3:
                        nc.vector.tensor_copy(
                            pks[3][0:la, bb, 1:1 + lv].bitcast(F32R),
                            cfp[:, 0:lv])
                    else:
                        cfs = mp.tile([la, lveff], F32, name=f"cf{i}_{bb}")
                        nc.vector.tensor_copy(cfs[:, :].bitcast(F32R),
                                              cfp[:, :])
                        cfss[(i, bb)] = cfs

            with tc.tile_pool(name="pyr", bufs=1) as wp:
                a_thunks = []
                v_thunks = []
                if "audio" in _STAGES:
                    a0_1 = new_act(wp, "a0c0o", 128, 515, 1)
                    lid0 = "a0c0"
                    bias0 = consts[0:128, meta["b"][lid0]:meta["b"][lid0] + 1]
                    ainbs = []
                    for bb in range(B):
                        ainb = wp.tile([80, 1, 2072], F32, name="ainb",
                                       tag="ainb", bufs=5)
                        eng = nc.sync if bb % 2 == 0 else nc.gpsimd
                        eng.dma_start(ainb[:, :, :].bitcast(F32R),
                                      af_in[0:80, bb:bb + 1, :].bitcast(F32R))
                        ainbs.append(ainb)

                    def a0c0_thunk(bb):
                        def th():
                            ainb = ainbs[bb]
                            for (l0, nmm_l, nout) in ((0, 258, 258),
                                                      (258, 258, 257)):
                                ps = pp.tile([128, 1, nmm_l], F32,
                                             name="a0c0_ps", tag="ps", bufs=7)
                                for dk in range(7):
                                    c0 = meta["w"][(lid0, 0, dk)]
                                    loff = l0 * 4 + dk
                                    rhs = ainb[0:80, 0:1,
                                               loff:loff + (nmm_l - 1) * 4 + 1:4]
                                    nc.tensor.matmul(
                                        ps[:, :, :],
                                        lhsT=_bc(consts[0:80, c0:c0 + 128]),
                                        rhs=_bc(rhs), start=(dk == 0),
                                        stop=(dk == 6))
                                nc.vector.tensor_scalar(
                                    a0_1[0:128, bb:bb + 1,
                                         1 + l0:1 + l0 + nout].bitcast(F32R),
                                    ps[:, :, 0:nout], bias0, 0.0,
                                    ALU.add, ALU.max)
                        return th
                    for bb in range(B):
                        a_thunks.append(a0c0_thunk(bb))

                    alens = [515, 257, 128, 64, 64]
                    a_state = {"cur": a0_1, "cpad": 1}

                    def a_conv_thunk(lid, cin, cout, Lo, out_name, pool_sel):
                        def th():
                            pool = mp if pool_sel else wp
                            nxt = new_act(pool, out_name, cout, Lo, 1)
                            conv(lid, [(a_state["cur"], 0, cin,
                                        a_state["cpad"])], Lo, nxt, 1,
                                 epi="dve")
                            a_state["cur"], a_state["cpad"] = nxt, 1
                            return nxt
                        return th

                    def a_stage_end(pi):
                        def th():
                            afeat[pi] = a_state["cur"]
                            if vfeat[pi] is not None:
                                emit_cf(pi)
                        return th

                    for j in range(1, 5):
                        a_thunks.append(a_conv_thunk(
                            f"a0c{j}", 128, 128, alens[j],
                            "af0" if j == 4 else f"a0x{j}", j == 4))
                    a_thunks.append(a_stage_end(0))
                    for pi in range(1, 4):
                        cin, cout = 128 >> (pi - 1), 128 >> pi
                        for j in range(3):
                            a_thunks.append(a_conv_thunk(
                                f"a{pi}c{j}", cin if j == 0 else cout, cout,
                                LA[pi], f"af{pi}" if j == 2 else f"a{pi}x{j}",
                                j == 2))
                        a_thunks.append(a_stage_end(pi))

                if "video" in _STAGES:
                    vin = wp.tile([80, B, 514], F32, name="vin")
                    nc.scalar.dma_start(vin[:, :, :].bitcast(F32R),
                                        vf_in[:, :, :].bitcast(F32R))
                    v_state = {"cur": vin, "cpad": 1}

                    def v_tile_thunk(lid, cin, cout, Lo, out_name, pool_sel,
                                     tile_i, holder):
                        def th():
                            if tile_i == 0:
                                pool = mp if pool_sel else wp
                                extra = {"vf3": 8, "vf0": 82}.get(out_name, 0)
                                holder["out"] = new_act(
                                    pool, out_name, cout, Lo, 1,
                                    padr_extra=extra)
                            conv(lid, [(v_state["cur"], 0, cin,
                                        v_state["cpad"])], Lo,
                                 holder["out"], 1, epi="act",
                                 b_range=conv_tiles(Lo)[tile_i])
                            if tile_i == len(conv_tiles(Lo)) - 1:
                                v_state["cur"], v_state["cpad"] = \
                                    holder["out"], 1
                        return th

                    def v_stage_end(pi):
                        def th():
                            vfeat[pi] = v_state["cur"]
                            if afeat[pi] is not None:
                                emit_cf(pi)
                        return th

                    def v_layer(lid, cin, cout, Lo, out_name, pool_sel):
                        holder = {}
                        for ti in range(len(conv_tiles(Lo))):
                            v_thunks.append(v_tile_thunk(
                                lid, cin, cout, Lo, out_name, pool_sel, ti,
                                holder))

                    for j in range(5):
                        v_layer(f"v0c{j}", 80 if j == 0 else 128, 128, LV[0],
                                "vf0" if j == 4 else f"v0x{j}", j == 4)
                    v_thunks.append(v_stage_end(0))
                    for pi in range(1, 4):
                        cin, cout = 128 >> (pi - 1), 128 >> pi
                        for j in range(3):
                            v_layer(f"v{pi}c{j}", cin if j == 0 else cout,
                                    cout, LV[pi],
                                    f"vf{pi}" if j == 2 else f"v{pi}x{j}",
                                    j == 2)
                        v_thunks.append(v_stage_end(pi))

                # rest of consts stream in behind the first chunks
                for ci in range(2, nchunk):
                    consts_chunk(ci, nc.sync if ci % 2 == 0 else nc.scalar)

                # ordering: the dense audio-c1 block first (DMA-paced), then
                # one audio layer : two video tiles (video fills the audio
                # chain's dependency stalls on the in-order PE)
                na0 = 8 if "audio" in _STAGES else 0
                for t in range(na0):
                    a_thunks[t]()
                ai, vi = na0, 0
                step_i = 0
                while ai < len(a_thunks) or vi < len(v_thunks):
                    if ai < len(a_thunks):
                        a_thunks[ai]()
                        ai += 1
                    nv = 3 if step_i < 7 else 1
                    for _ in range(nv):
                        if vi < len(v_thunks):
                            v_thunks[vi]()
                            vi += 1
                    step_i += 1

            # ---------------- decoder ----------------
            if "decoder" in _STAGES:
             with tc.tile_pool(name="dec", bufs=1) as dw:

                def transposed_flow(tag, flow_lb, Lf):
                    """flow (l-major [1, Lf, B]) -> [rows, B] chunks, one
                    partition-fold DMA each."""
                    fts = []
                    for c in range((Lf + 127) // 128):
                        rows = min(128, Lf - 128 * c)
                        ft = dw.tile([rows, B], F32, name=f"ft{tag}_{c}")
                        nc.sync.dma_start(
                            ft[:, :].bitcast(F32R),
                            flow_lb[0:1, 128 * c:128 * c + rows, :].bitcast(F32R))
                        fts.append((ft, rows))
                    return fts

                def upsample(fts, uname, lo):
                    lo_eff = lo + (lo & 1)
                    ps = pp.tile([B, lo_eff], F32, name=f"up_{uname}",
                                 tag="ps", bufs=7)
                    chunks = meta["U"][uname]
                    for ci, ((rows, c0), (ft, _r)) in enumerate(
                            zip(chunks, fts)):
                        nc.tensor.matmul(
                            ps[:, :], lhsT=_bc(ft[0:rows, 0:B]),
                            rhs=_bc(consts[0:rows, c0:c0 + lo_eff]),
                            start=(ci == 0), stop=(ci == len(chunks) - 1))
                    return ps

                def emit_warp(i, G):
                    """Apply bilinear warp G to the stored correlations;
                    write into pack rows [0:la]."""
                    la, lv = LA[i], LV[i]
                    lveff = 256 if i == 0 else lv + (lv & 1)
                    pk = pks[i]
                    for bb in range(B):
                        crp = pp.tile([la, lveff], F32, name="crp", tag="ps",
                                      bufs=7)
                        nc.tensor.matmul(crp[:, :],
                                         lhsT=_bc(G[0:la, bb, 0:la]),
                                         rhs=_bc(cfss[(i, bb)][:, :]),
                                         start=True, stop=True)
                        nc.vector.tensor_copy(
                            pk[0:la, bb, 1:1 + lv].bitcast(F32R),
                            crp[:, 0:lv])

                upch = {}
                Gs = {}

                def emit_flow_products(i, flow_lb):
                    """After flow_i (l-major): audio upsample -> G[i-1]
                    (critical: gates the next level's warp), then video
                    upsample -> out_i (+ upch[i-1])."""
                    fts = transposed_flow(i, flow_lb, LV[i])
                    if i != 0:
                        j = i - 1
                        la = LA[j]
                        ua = upsample(fts, f"ua{j}", la)
                        ixs = dw.tile([B, la], F32, name=f"ixs{j}")
                        nc.scalar.copy(ixs[:, :], ua[:, 0:la])
                        ixt = dw.tile([1, B, la], F32, name=f"ixt{j}")
                        nc.sync.dma_start(ixt[0:1, :, :].bitcast(F32R),
                                          ixs[0:B, 0:la].bitcast(F32R))
                        gps = pp.tile([la, B, la], F32, name=f"gps{j}",
                                      tag="ps", bufs=7)
                        oc = meta["ones"]
                        nc.tensor.matmul(gps[:, :, :],
                                         lhsT=_bc(consts[0:1, oc:oc + la]),
                                         rhs=_bc(ixt[0:1, :, :]),
                                         start=True, stop=True)
                        gt = dw.tile([la, B, la], F32, name="gtmp",
                                     tag="gtmp", bufs=2)
                        gbc = meta["gb"][f"gb{j}"]
                        nc.scalar.activation(gt[:, :, :], gps[:, :, :],
                                             AF.Abs,
                                             bias=consts[0:la, gbc:gbc + 1])
                        G = dw.tile([la, B, la], F32, name=f"G{j}")
                        nc.scalar.activation(G[:, :, :].bitcast(F32R),
                                             gt[:, :, :], AF.Relu,
                                             bias=1.0, scale=-1.0)
                        Gs[j] = G
                        emit_warp(j, G)
                    lo = VUP[i]
                    ups = upsample(fts, f"u{i}", lo)
                    usb = dw.tile([B, lo], F32, name=f"usb{i}", tag="usb",
                                  bufs=2)
                    nc.scalar.copy(usb[:, :], ups[:, 0:lo])
                    nc.gpsimd.dma_start(out_d[i][:, :], usb[:, :])
                    if i != 0:
                        j = i - 1
                        # 3 rows, one per conv tap, pre-shifted by one column
                        uc = dw.tile([3, B, lo + 6], F32, name=f"upch{j}")
                        nc.gpsimd.memset(uc[0:3, :, 0:2], 0.0)
                        nc.gpsimd.memset(uc[0:3, :, lo:lo + 6], 0.0)
                        for r, eng in enumerate((nc.sync, nc.scalar,
                                                 nc.gpsimd)):
                            eng.dma_start(
                                uc[r:r + 1, :, 2 - r:2 - r + lo].bitcast(F32R),
                                usb[0:B, 0:lo].bitcast(F32R))
                        upch[j] = uc

                flows = {}
                for i in (3, 2, 1, 0):
                    la, cv, cadd = LA[i], 128 >> i, 32 >> i
                    AE, BE = PKA[i]
                    pk = pks[i]
                    gv = (vfeat[i], 0, cv, 1)
                    gup = [] if i == 3 else [("tap3", upch[i], 1)]
                    fl3 = 32 if i == 3 else None
                    conv(f"e{i}a", [(pk, 0, la, 1), gv] + gup, LV[i], pk, 1,
                         epi="act", out_r0=AE, force_leff=fl3)
                    conv(f"e{i}b", [(pk, 0, AE + cadd, 1), gv] + gup, LV[i],
                         pk, 1, epi="act", out_r0=BE, force_leff=fl3)
                    gpk = (pk, 0, BE + cadd, 1)
                    fl = dw.tile([1, LV[i], B], F32, name=f"flow{i}")
                    conv(f"p{i}", [gpk, gv] + gup, LV[i], fl, 0, epi="act",
                         out_lmajor=True, force_leff=fl3)
                    flows[i] = fl
                    if i != 0:
                        emit_flow_products(i, fl)
                    else:
                        dp_groups = [gpk, gv] + gup

                # dp chain
                dpads = [2, 4, 8, 0]
                douts = [64, 16, 4, 1]
                for j in range(4):
                    pad = dpads[j]
                    if j == 3:
                        t = dw.tile([1, 170, B], F32, name="dpb3")
                        conv("dp3", dp_groups, 170, t, 0, epi="dve",
                             out_lmajor=True)
                    else:
                        t = new_act(dw, f"dpb{j}", douts[j], 170, pad)
                        conv(f"dp{j}", dp_groups, 170, t, pad, epi="dve")
                    dp_groups = [(t, 0, douts[j], pad)]
                dp4 = dp_groups[0][0]
                flF = dw.tile([1, 170, B], F32, name="flF")
                nc.vector.tensor_add(flF[0:1, :, :], flows[0][0:1, :, :],
                                     dp4[0:1, :, :])
                emit_flow_products(0, flF)

    nc.finalize()
    return nc, spec, meta


_CACHE = {}


def _get_graph():
    if "nc" not in _CACHE:
        nc, spec, meta = _build()
        _CACHE["nc"] = (nc, spec, meta)
    return _CACHE["nc"]


def _prep_core_inputs(vf8, af8, consts):
    """vf8 [8,512,20,2,2], af8 [8,80,2056] -> device layouts."""
    v = np.ascontiguousarray(
        vf8.reshape(B, 512, 80).transpose(2, 0, 1)).astype(np.float32)
    vp = np.zeros((80, B, 514), np.float32)
    vp[:, :, 1:513] = v
    a = np.ascontiguousarray(af8.transpose(1, 0, 2)).astype(np.float32)
    ap_ = np.zeros((80, B, 2072), np.float32)
    ap_[:, :, 4:2060] = a
    return {"vf": vp, "af": ap_, "consts": consts}


def kernel(video_feature, audio_feature, params):
    nc, spec, meta = _get_graph()
    consts = _pack_consts(params, spec, meta)
    vf = np.asarray(video_feature, np.float32)
    af = np.asarray(audio_feature, np.float32)
    in_maps = [
        _prep_core_inputs(vf[c * B:(c + 1) * B], af[c * B:(c + 1) * B], consts)
        for c in range(NCORES)
    ]
    res = run_bass_kernel_spmd(nc, in_maps, core_ids=list(range(NCORES)))
    outs = tuple(
        np.concatenate([res.results[c][f"out{k}"] for c in range(NCORES)], axis=0)
        for k in range(4))
    return outs
